# revision 6
# baseline (speedup 1.0000x reference)
"""Trainium2 Bass kernel for single-head DotProductAttention with softmax over
the *query* axis (axis=-2) and causal mask, returning (out, attention).

Reference semantics (B=4, S=2048, D_MODEL=1024, D_K=D_V=128):
    Q = x @ w_q; K = x @ w_k; V = x @ w_v
    scores[b,q,k] = (Q[b,q] . K[b,k]) / sqrt(128),  masked to -inf for k > q
    attention = softmax(scores, axis=-2)            # normalize over q per column k
    out[b,q,v] = sum_k attention[b,q,k] V[b,k,v]

Sharding: 8 cores = (batch b, half h). Core (b,h) owns 8 interleaved k-chunks
of 128 columns at global k0 = 256*j + 128*h (j=0..7), which balances the
causal triangle across the pair of cores sharing a batch.

Device layout: scores are computed transposed, S^T[k_part, q_free], so the
softmax over q is a free-axis reduction (per-partition max / fused exp+accum
on the scalar engine). Causality: chunk j only computes q >= 256*j. The
128x256 diagonal-block mask is a data input so the program is SPMD-uniform.
The attention output is stored chunk-major *transposed* ([k, q]); the host
transposes while scattering into the full [B,S,S] array. out is accumulated
as O^T = sum_j (V_j/denom_j)^T-weighted A~^T directly in PSUM.

Host does layout-only work (transpose/gather/scatter/scale); all matmul
FLOPs stay on device in fp32.
"""

import sys
from contextlib import ExitStack

import numpy as np

if "/opt/trn_rl_repo" not in sys.path:
    sys.path.insert(0, "/opt/trn_rl_repo")

import concourse.bass as bass
import concourse.tile as tile
from concourse import bacc, mybir
from concourse.bass_utils import run_bass_kernel_spmd
from concourse.masks import make_identity

B, S, DM, DK, DV = 4, 2048, 1024, 128, 128
NCHUNK = 8          # k-chunks per core, 128 wide
CW = 128            # chunk width
QSTEP = 256         # q start of chunk j is QSTEP*j
INV_SQRT_DK = 1.0 / np.sqrt(np.float32(DK))
NEG = -1.0e30

_CACHE = {}


def _build_program():
    f32 = mybir.dt.float32
    nc = bacc.Bacc("TRN2", target_bir_lowering=False, debug=False, num_devices=8)

    xt = nc.dram_tensor("xt", [DM, S], f32, kind="ExternalInput").ap()
    xk = nc.dram_tensor("xk", [DM, NCHUNK * CW], f32, kind="ExternalInput").ap()
    wq = nc.dram_tensor("wq", [DM, DK], f32, kind="ExternalInput").ap()
    wk = nc.dram_tensor("wk", [DM, DK], f32, kind="ExternalInput").ap()
    wv = nc.dram_tensor("wv", [DM, DV], f32, kind="ExternalInput").ap()
    dmask = nc.dram_tensor("dmask", [CW, QSTEP], f32, kind="ExternalInput").ap()
    # A~^T chunk-major: rows j*128..(j+1)*128 = chunk j (k within chunk),
    # cols = q. Only q >= 256*j is written; the rest stays zero.
    attn = nc.dram_tensor("attn", [NCHUNK * CW, S], f32, kind="ExternalOutput").ap()
    ot = nc.dram_tensor("ot", [DV, S], f32, kind="ExternalOutput").ap()
    recip_out = nc.dram_tensor("recip", [CW, NCHUNK], f32, kind="ExternalOutput").ap()

    MC = DM // 128  # 8 contraction chunks over d_model

    with tile.TileContext(nc) as tc, ExitStack() as ctx:
        pers = ctx.enter_context(tc.tile_pool(name="pers", bufs=1))
        QT = pers.tile([128, S], f32, name="QT", tag="QT")
        KT = pers.tile([128, NCHUNK * CW], f32, name="KT", tag="KT")
        Vt = [pers.tile([128, DV], f32, name=f"V{j}", tag=f"V{j}")
              for j in range(NCHUNK)]
        AT = [pers.tile([128, S - QSTEP * j], f32, name=f"AT{j}", tag=f"AT{j}")
              for j in range(NCHUNK)]
        RECIP = pers.tile([128, NCHUNK], f32, name="RECIP", tag="RECIP")
        DEN = pers.tile([128, NCHUNK], f32, name="DEN", tag="DEN")
        OT = pers.tile([128, S], f32, name="OT", tag="OT")
        dm = pers.tile([CW, QSTEP], f32, name="dmask", tag="dmask")
        ident = pers.tile([128, 128], f32, name="ident", tag="ident")

        # ---------------- phase B: projections ----------------
        with tc.tile_pool(name="pB", bufs=1) as pB, \
             tc.tile_pool(name="psB", bufs=2, space="PSUM") as psB:
            # DMA issue order = compute consumption order: wq, xt seg0, then
            # the rest. Q's first matmul can start after ~2.5MB arrives.
            WQ = []
            for m in range(MC):
                t = pB.tile([128, 128], f32, name=f"wq{m}", tag=f"wq{m}")
                nc.sync.dma_start(t[:], wq[m * 128:(m + 1) * 128, :])
                WQ.append(t)
            XT = {}
            for s in range(4):
                for m in range(MC):
                    t = pB.tile([128, 512], f32, name=f"xt{m}_{s}", tag=f"xt{m}_{s}")
                    nc.sync.dma_start(t[:], xt[m * 128:(m + 1) * 128,
                                               s * 512:(s + 1) * 512])
                    XT[(m, s)] = t
            WK, WV = [], []
            for m in range(MC):
                t = pB.tile([128, 128], f32, name=f"wk{m}", tag=f"wk{m}")
                nc.sync.dma_start(t[:], wk[m * 128:(m + 1) * 128, :])
                WK.append(t)
            XK = {}
            for s in range(2):
                for m in range(MC):
                    t = pB.tile([128, 512], f32, name=f"xk{m}_{s}", tag=f"xk{m}_{s}")
                    nc.sync.dma_start(t[:], xk[m * 128:(m + 1) * 128,
                                               s * 512:(s + 1) * 512])
                    XK[(m, s)] = t
            for m in range(MC):
                t = pB.tile([128, 128], f32, name=f"wv{m}", tag=f"wv{m}")
                nc.sync.dma_start(t[:], wv[m * 128:(m + 1) * 128, :])
                WV.append(t)
            nc.sync.dma_start(dm[:], dmask[:, :])
            make_identity(nc, ident[:])

            # Q^T[d, q] = sum_m wq[m, d] * xt[m, q]
            for s in range(4):
                ps = psB.tile([128, 512], f32, name="psB", tag="psB")
                for m in range(MC):
                    nc.tensor.matmul(ps[:], WQ[m][:], XT[(m, s)][:],
                                     start=(m == 0), stop=(m == MC - 1))
                nc.scalar.copy(QT[:, s * 512:(s + 1) * 512], ps[:])
            # K^T[d, kl]
            for s in range(2):
                ps = psB.tile([128, 512], f32, name="psB", tag="psB")
                for m in range(MC):
                    nc.tensor.matmul(ps[:], WK[m][:], XK[(m, s)][:],
                                     start=(m == 0), stop=(m == MC - 1))
                nc.scalar.copy(KT[:, s * 512:(s + 1) * 512], ps[:])
            # V^T[v, kl] then transpose per chunk to V[kl, v]
            VTT = pers.tile([128, NCHUNK * CW], f32, name="VTT", tag="VTT")
            for s in range(2):
                ps = psB.tile([128, 512], f32, name="psB", tag="psB")
                for m in range(MC):
                    nc.tensor.matmul(ps[:], WV[m][:], XK[(m, s)][:],
                                     start=(m == 0), stop=(m == MC - 1))
                nc.scalar.copy(VTT[:, s * 512:(s + 1) * 512], ps[:])
            for j in range(NCHUNK):
                ps = psB.tile([128, 512], f32, name="psB", tag="psB")
                nc.tensor.transpose(ps[:, 0:128],
                                    VTT[:, j * CW:(j + 1) * CW], ident[:])
                nc.vector.tensor_copy(Vt[j][:], ps[:, 0:128])

        # ---------------- phases C (scores/softmax) + D (out) ----------------
        with tc.tile_pool(name="pC", bufs=2) as pC, \
             tc.tile_pool(name="psS", bufs=2, space="PSUM") as psS, \
             tc.tile_pool(name="psO", bufs=1, space="PSUM") as psO:

            pso = psO.tile([128, S], f32, name="O", tag="O")

            for j in range(NCHUNK):
                qs0 = QSTEP * j
                lhs = KT[:, j * CW:(j + 1) * CW]
                maxs = pC.tile([128, 8], f32, name="maxs", tag="maxs")
                nseg = 0
                for qs in range(qs0, S, 512):
                    w = min(512, S - qs)
                    ps = psS.tile([128, 512], f32, name="S", tag="S")
                    nc.tensor.matmul(ps[:, 0:w], lhs, QT[:, qs:qs + w],
                                     start=True, stop=True)
                    if qs == qs0:
                        nc.vector.tensor_add(ps[:, 0:QSTEP], ps[:, 0:QSTEP],
                                             dm[:])
                    nc.vector.reduce_max(maxs[:, nseg:nseg + 1], ps[:, 0:w],
                                         axis=mybir.AxisListType.X)
                    nc.scalar.copy(AT[j][:, qs - qs0:qs - qs0 + w], ps[:, 0:w])
                    nseg += 1
                m = pC.tile([128, 1], f32, name="m", tag="m")
                nc.vector.reduce_max(m[:], maxs[:, 0:nseg],
                                     axis=mybir.AxisListType.X)
                bias = pC.tile([128, 1], f32, name="bias", tag="bias")
                nc.scalar.mul(bias[:], m[:], -INV_SQRT_DK)
                nc.scalar.activation(AT[j][:], AT[j][:],
                                     mybir.ActivationFunctionType.Exp,
                                     bias=bias[:], scale=float(INV_SQRT_DK),
                                     accum_out=DEN[:, j:j + 1])
                nc.vector.reciprocal(RECIP[:, j:j + 1], DEN[:, j:j + 1])

                # store A~^T chunk rows directly (host transposes)
                nc.sync.dma_start(attn[j * CW:(j + 1) * CW, qs0:S], AT[j][:])

                # inline O^T accumulation for this chunk
                vs = pers.tile([128, DV], f32, name=f"VS{j}", tag=f"VS{j}")
                nc.vector.tensor_scalar_mul(vs[:], Vt[j][:], RECIP[:, j:j + 1])
                # pieces aligned to 512-wide PSUM banks
                pieces = []
                qs = qs0
                if qs % 512:
                    pieces.append((qs, 512 - qs % 512))
                    qs += 512 - qs % 512
                while qs < S:
                    pieces.append((qs, min(512, S - qs)))
                    qs += 512
                for qs, w in pieces:
                    nc.tensor.matmul(pso[:, qs:qs + w], vs[:],
                                     AT[j][:, qs - qs0:qs - qs0 + w],
                                     start=(j == 0), stop=(j == NCHUNK - 1),
                                     skip_group_check=True)

            nc.sync.dma_start(recip_out[:, :], RECIP[:])
            nc.scalar.copy(OT[:], pso[:])
            nc.sync.dma_start(ot[:, :], OT[:])

    nc.compile()
    return nc


def _get_program():
    if "nc" not in _CACHE:
        _CACHE["nc"] = _build_program()
    return _CACHE["nc"]


def _core_inputs(x_q, w_q, w_k, w_v, c):
    b, h = divmod(c, 2)
    xt = np.ascontiguousarray(x_q[b].T)
    cols = np.concatenate(
        [np.arange(QSTEP * j + CW * h, QSTEP * j + CW * h + CW)
         for j in range(NCHUNK)])
    xk = np.ascontiguousarray(xt[:, cols])
    p = np.arange(CW)
    ql = np.arange(QSTEP)
    dmask = np.where(ql[None, :] >= (CW * h + p)[:, None],
                     np.float32(0.0), np.float32(NEG)).astype(np.float32)
    return {"xt": xt, "xk": xk, "wq": w_q, "wk": w_k, "wv": w_v,
            "dmask": dmask}


def kernel(x_q, w_q, w_k, w_v, _trace=False, _results_hook=None):
    x_q = np.ascontiguousarray(x_q, dtype=np.float32)
    w_q = np.ascontiguousarray(w_q, dtype=np.float32)
    w_k = np.ascontiguousarray(w_k, dtype=np.float32)
    w_v = np.ascontiguousarray(w_v, dtype=np.float32)

    nc = _get_program()
    in_maps = [_core_inputs(x_q, w_q, w_k, w_v, c) for c in range(8)]
    res = run_bass_kernel_spmd(nc, in_maps, list(range(8)), trace=_trace)
    if _results_hook is not None:
        _results_hook(res)

    attention = np.zeros((B, S, S), dtype=np.float32)
    out = np.empty((B, S, DV), dtype=np.float32)
    for b in range(B):
        o = None
        for h in range(2):
            c = 2 * b + h
            r = res.results[c]
            a_raw = r["attn"]                 # [1024, S] chunk-major, [k, q]
            rc = r["recip"]                   # [128, 8]
            for j in range(NCHUNK):
                k0 = QSTEP * j + CW * h
                qs0 = QSTEP * j
                blk = a_raw[j * CW:(j + 1) * CW, qs0:] * rc[:, j][:, None]
                attention[b][qs0:, k0:k0 + CW] = blk.T
            oc = r["ot"]                      # [DV, S]
            o = oc if o is None else o + oc
        out[b] = o.T
    return out, attention
